# revision 24
# baseline (speedup 1.0000x reference)
"""Attentional pooling layer on Trainium2 (Bass/Tile), 8-core batch-parallel.

Reference computation per batch b:
    scores[hw, n] = sum_c f[c, hw] * w[c, n]          (mm1, fp16 in, f32 acc)
    num           = softplus(scores)                  (ACT: Abs/Exp/Ln, f32)
    denom[n]      = sum_hw num[hw, n] + 16*CONST      (PE reduce + DVE)
    att[hw, n]    = (num + CONST) / denom[n]          (PE bcast + DVE, fp16)
    out[c, n]     = sum_hw f[c, hw] * att[hw, n]      (mm2, fp16 in, f32 acc)

The problem is HBM-bandwidth-bound: per core 32 batches x 2 MiB of weights
in and 2 MiB of outputs out.  Both streams run in fp16 (tolerance is 2e-2;
fp16 end-to-end lands ~5e-4), halving DMA traffic vs fp32.  Weights are
converted to fp16 on the host; outputs are written fp16 by the PSUM->SBUF
copy and upconverted on the host.

Partition layout: 4 batches are packed into one full 128-partition group at
32-partition offsets (fp16 matmuls accept explicit tile_position col bases
0/32/64/96, unlike f32r which requires base 0).  32 batches per core = 8
exact groups of 4.  Partition-dim reductions (sum over hw) and broadcasts
(denom over hw) are tiny constant 0/1 f32r matmuls (bd / exp4) fed from
host numpy.  Features arrive twice from the host, both fp16 and tiny: once
c-major zero-padded for mm1 (fpad) and once pre-transposed hw-major for
mm2 (ftg), which kills the on-device PE transposes and frees a PSUM bank.

Scores land in two 2-bank PSUM tiles per group so the softplus pointwise
ops run on [128, 1024] (half the instruction count).  The weight pool
holds 12 tiles (3 groups of prefetch): deeper prefetch front-loads the
shared DMA engines and delays the output stores; shallower starves mm1.

DMA queues: weight/feature loads issue on SP (HWDGE) and output stores on
the otherwise-idle Pool engine (SWDGE), so a store waiting on its tile
never head-of-line blocks the weight prefetch stream.  PSUM->SBUF output
copies (the fp32->fp16 conversion) split over ACT and DVE (GPSIMD cannot
read PSUM).
"""

import numpy as np
from contextlib import ExitStack

import concourse.bass as bass
import concourse.bacc as bacc
import concourse.tile as tile
from concourse import mybir
from concourse.bass_utils import run_bass_kernel_spmd

F32 = mybir.dt.float32
F32R = mybir.dt.float32r
F16 = mybir.dt.float16
AF = mybir.ActivationFunctionType
ALU = mybir.AluOpType

N_CORES = 8
B_FULL, C, H, W, N = 256, 256, 4, 4, 2048
HW = H * W                  # 16
B = B_FULL // N_CORES       # 32 batches per core
KC = C // 128               # 2 contraction chunks of 128
GB = 4                      # batches per partition group (offsets 0/32/64/96)
GP = 32 * GB                # 128 partitions per group
NG = B // GB                # 8 groups per core
NCH = 4                     # n chunks per group chain
NW = N // NCH               # 512 (one PSUM bank)
NH = N // 2                 # 1024 (half-N weight tile)
CONST = 1e-4


def aux_inputs():
    # bd[p, j] = 1 iff row p is one of batch-slot j's real hw rows
    bd = np.zeros((GP, GB), np.float32)
    for p in range(GP):
        if p % 32 < HW:
            bd[p, p // 32] = 1.0
    # exp4[j, p] = 1 iff partition p belongs to batch-slot j's 32-block
    exp4 = np.zeros((GB, GP), np.float32)
    for p in range(GP):
        exp4[p // 32, p] = 1.0
    return {"bd": bd, "exp4": exp4}


def build_nc(debug=False):
    nc = bacc.Bacc(None, target_bir_lowering=False, debug=debug)
    feat = nc.dram_tensor("fpad", [128, KC, B, 32], F16, kind="ExternalInput")
    ftg_d = nc.dram_tensor("ftg", [NG, GP, KC, 128], F16, kind="ExternalInput")
    wts = nc.dram_tensor("weights", [B, C, N], F16, kind="ExternalInput")
    out = nc.dram_tensor("out", [B, C, N], F16, kind="ExternalOutput")
    bd_d = nc.dram_tensor("bd", [GP, GB], F32R, kind="ExternalInput")
    exp_d = nc.dram_tensor("exp4", [GB, GP], F32R, kind="ExternalInput")

    # [ci, b, kc, n] views of the DRAM tensors
    wts_r = wts.ap().rearrange("b (kc ci) n -> ci b kc n", kc=KC)
    out_r = out.ap().rearrange("b (kc ci) n -> ci b kc n", kc=KC)

    with tile.TileContext(nc) as tc, ExitStack() as ctx:
        singles = ctx.enter_context(tc.tile_pool(name="singles", bufs=1))
        wpool = ctx.enter_context(tc.tile_pool(name="w", bufs=12))
        opool = ctx.enter_context(tc.tile_pool(name="o", bufs=4))
        numpool = ctx.enter_context(tc.tile_pool(name="num", bufs=3))
        attpool = ctx.enter_context(tc.tile_pool(name="att", bufs=2))
        smallpool = ctx.enter_context(tc.tile_pool(name="small", bufs=3))
        ftpool = ctx.enter_context(tc.tile_pool(name="ft", bufs=6))
        ps_sc = ctx.enter_context(tc.tile_pool(name="ps_sc", bufs=2, space="PSUM"))
        ps_dr = ctx.enter_context(tc.tile_pool(name="ps_dr", bufs=2, space="PSUM"))
        ps_o = ctx.enter_context(tc.tile_pool(name="ps_o", bufs=2, space="PSUM"))

        bd_t = singles.tile([GP, GB], F32R)
        nc.sync.dma_start(out=bd_t, in_=bd_d.ap())
        exp_t = singles.tile([GB, GP], F32R)
        nc.sync.dma_start(out=exp_t, in_=exp_d.ap())

        # features, pre-transposed + hw-padded to 32 with zeros on the host
        f_t = singles.tile([128, KC, B, 32], F16)
        nc.sync.dma_start(out=f_t, in_=feat.ap())

        st = {}
        ev = 0

        def issue_loads(g):
            bs = list(range(g * GB, (g + 1) * GB))
            w_t = {}
            for b in bs:
                w_t[b] = wpool.tile([128, KC, N], F16, tag="w", name="w_t")
                nc.sync.dma_start(out=w_t[b], in_=wts_r[:, b])
            ft_t = ftpool.tile([GP, KC, 128], F16, name="ft_t")
            nc.sync.dma_start(out=ft_t, in_=ftg_d.ap()[g])
            st[g] = {"bs": bs, "ft": ft_t, "w": w_t}

        def issue_mm1(g):
            # scores land in two 2-bank PSUM tiles [128, 2, 512] so the
            # pointwise softplus ops run on [128, 1024] (half the op count)
            s = st[g]
            sc_l = [ps_sc.tile([GP, 2, NW], F32, name="sc_ps") for _ in range(2)]
            for nb in range(NCH):
                for j in range(GB):
                    for kc in range(KC):
                        nc.tensor.matmul(
                            sc_l[nb // 2][32 * j : 32 * j + 32, nb % 2, :],
                            f_t[:, kc, s["bs"][j], :],
                            s["w"][s["bs"][j]][:, kc, nb * NW : (nb + 1) * NW],
                            start=(kc == 0),
                            stop=(kc == KC - 1),
                            tile_position=(0, 32 * j),
                        )
            s["sc"] = sc_l

        def issue_softplus(g):
            # softplus(x) = max(x,0) + ln(1 + exp(-|x|)): exp arg <= 0 so no
            # overflow, Ln input stays in [1,2].  All Abs+Exp, then all Lns,
            # so the ACT table-set switches twice per group, not per chunk.
            s = st[g]
            te_l, num_l = [], []
            exp_insts = []
            for p in range(2):
                t_abs = numpool.tile([GP, 2, NW], F32, tag="tabs", bufs=2)
                nc.scalar.activation(t_abs, s["sc"][p], AF.Abs)
                t_exp = numpool.tile([GP, 2, NW], F32, tag="texp", bufs=2)
                exp_insts.append(
                    nc.scalar.activation(t_exp, t_abs, AF.Exp, scale=-1.0)
                )
                te_l.append(t_exp)
            for p in range(2):
                t_ln = numpool.tile([GP, 2, NW], F32, tag="tln", bufs=2)
                ln_i = nc.scalar.activation(t_ln, te_l[p], AF.Ln, bias=1.0)
                tile.add_dep_helper(
                    ln_i.ins, exp_insts[-1].ins, sync=False,
                    reason="cluster Lns after all Exps (one table switch)",
                )
                num_t = numpool.tile([GP, 2, NW], F32R, tag="num", bufs=2)
                nc.vector.scalar_tensor_tensor(
                    num_t, s["sc"][p], 0.0, t_ln, op0=ALU.max, op1=ALU.add
                )
                num_l.append(num_t)
            s["num"] = num_l

        def issue_att(g):
            s = st[g]
            att_t = attpool.tile([GP, NCH, NW], F16)
            for nb in range(NCH):
                num_nb = s["num"][nb // 2][:, nb % 2, :]
                d_ps = ps_dr.tile([GB, NW], F32, tag="dr", name="d_ps")
                nc.tensor.matmul(d_ps, bd_t, num_nb, start=True, stop=True)
                r_t = smallpool.tile([GB, NW], F32R)
                with nc.allow_low_precision(reason="tf32 matmul operand"):
                    nc.vector.tensor_scalar_add(r_t, d_ps, HW * CONST)
                    nc.vector.reciprocal(r_t, r_t)
                rb_ps = ps_dr.tile([GP, NW], F32, tag="dr", name="rb_ps")
                nc.tensor.matmul(rb_ps, exp_t, r_t, start=True, stop=True)
                # att = (num + CONST) * (1/denom), written fp16 for mm2
                with nc.allow_low_precision(reason="fp16 mm2 operand"):
                    nc.vector.scalar_tensor_tensor(
                        att_t[:, nb, :],
                        num_nb,
                        CONST,
                        rb_ps,
                        op0=ALU.add,
                        op1=ALU.mult,
                    )
            s["att"] = att_t

        def issue_mm2(g):
            # chunk-pair-major inner order: the first four matmuls of each
            # batch depend only on the first att pair
            s = st[g]
            att_t = s["att"]
            s["o_ps"] = []
            for j in range(GB):
                o_sb = opool.tile([128, KC, N], F16, tag="o", name="o_sb")
                for kc in range(KC):
                    for nb in range(NCH):
                        o_ps = ps_o.tile([128, NW], F32)
                        nc.tensor.matmul(
                            o_ps,
                            s["ft"][32 * j : 32 * j + HW, kc, :],
                            att_t[32 * j : 32 * j + HW, nb, :],
                            start=True,
                            stop=True,
                            tile_position=(32 * j, 0),
                        )
                        s["o_ps"].append((j, kc, nb, o_ps, o_sb))

        def issue_out(g):
            nonlocal ev
            s = st[g]
            for j, kc, nb, o_ps, o_sb in s["o_ps"]:
                dst = o_sb[:, kc, nb * NW : (nb + 1) * NW]
                with nc.allow_low_precision(reason="fp16 output"):
                    # GPSIMD can't read PSUM; ACT-heavy split since ACT
                    # copies are cheaper and DVE carries the denominator/
                    # attention elementwise chain.
                    if ev % 16 < 9:
                        nc.scalar.copy(dst, o_ps)
                    else:
                        nc.vector.tensor_copy(dst, o_ps)
                ev += 1
                if kc == KC - 1 and nb == NCH - 1:
                    nc.gpsimd.dma_start(out=out_r[:, s["bs"][j]], in_=o_sb)
            del st[g]

        # Pipeline: copies+stores of group g-1 issue AFTER softplus(g), so
        # the ACT queue never parks on an mm2-gated copy while softplus work
        # (which gates the whole denominator chain) is available.
        for g in range(NG):
            issue_loads(g)
            issue_mm1(g)
            if g >= 1:
                issue_att(g - 1)
                issue_mm2(g - 1)
            issue_softplus(g)
            if g >= 1:
                issue_out(g - 1)
        issue_att(NG - 1)
        issue_mm2(NG - 1)
        issue_out(NG - 1)

    nc.compile()
    return nc


_NC_CACHE = {}


def _get_nc():
    if "nc" not in _NC_CACHE:
        _NC_CACHE["nc"] = build_nc()
    return _NC_CACHE["nc"]


def prep_features(features):
    """[B_FULL, C, H, W] f32 -> (fpad [128, KC, B_FULL, 32] f16,
    ftg [B_FULL//GB, GP, KC, 128] f16)."""
    features = np.asarray(features, dtype=np.float32)
    f4 = features.reshape(B_FULL, KC, 128, HW)
    fpad = np.zeros((B_FULL, KC, 128, 32), np.float16)
    fpad[..., :HW] = f4
    fpad = np.ascontiguousarray(fpad.transpose(2, 1, 0, 3))  # [128, KC, b, 32]
    # hw-major for mm2: ftg[g, 32j+r, kc, ci] = f[4g+j, kc*128+ci, r]
    ftp = np.zeros((B_FULL, 32, KC, 128), np.float16)
    ftp[:, :HW] = f4.transpose(0, 3, 1, 2)
    ftg = np.ascontiguousarray(ftp.reshape(B_FULL // GB, GP, KC, 128))
    return fpad, ftg


def run(features, weights, trace=False, **kwargs):
    """Shard over 8 cores, run, gather. Returns (out, BassKernelResults)."""
    fpad, ftg = prep_features(features)
    weights = np.asarray(weights).astype(np.float16)
    aux = aux_inputs()
    nc = _get_nc()
    in_maps = []
    for i in range(N_CORES):
        sl = slice(i * B, (i + 1) * B)
        gl = slice(i * NG, (i + 1) * NG)
        in_maps.append(
            {
                "fpad": np.ascontiguousarray(fpad[:, :, sl]),
                "ftg": ftg[gl],
                "weights": weights[sl],
                **aux,
            }
        )
    res = run_bass_kernel_spmd(
        nc, in_maps, core_ids=list(range(N_CORES)), trace=trace, **kwargs
    )
    out = np.concatenate([r["out"] for r in res.results], axis=0).astype(np.float32)
    return out, res


def kernel(features, weights):
    out, _ = run(features, weights)
    return out


# revision 25
# speedup vs baseline: 1.0005x; 1.0005x over previous
"""Attentional pooling layer on Trainium2 (Bass/Tile), 8-core batch-parallel.

Reference computation per batch b:
    scores[hw, n] = sum_c f[c, hw] * w[c, n]          (mm1, fp16 in, f32 acc)
    num           = softplus(scores)                  (ACT: Abs/Exp/Ln, f32)
    denom[n]      = sum_hw num[hw, n] + 16*CONST      (PE reduce + DVE)
    att[hw, n]    = (num + CONST) / denom[n]          (PE bcast + DVE, fp16)
    out[c, n]     = sum_hw f[c, hw] * att[hw, n]      (mm2, fp16 in, f32 acc)

The problem is HBM-bandwidth-bound: per core 32 batches x 2 MiB of weights
in and 2 MiB of outputs out.  Both streams run in fp16 (tolerance is 2e-2;
fp16 end-to-end lands ~5e-4), halving DMA traffic vs fp32.  Weights are
converted to fp16 on the host; outputs are written fp16 by the PSUM->SBUF
copy and upconverted on the host.

Partition layout: 4 batches are packed into one full 128-partition group at
32-partition offsets (fp16 matmuls accept explicit tile_position col bases
0/32/64/96, unlike f32r which requires base 0).  32 batches per core = 8
exact groups of 4.  Partition-dim reductions (sum over hw) and broadcasts
(denom over hw) are tiny constant 0/1 f32r matmuls (bd / exp4) fed from
host numpy.  Features arrive twice from the host, both fp16 and tiny: once
c-major zero-padded for mm1 (fpad) and once pre-transposed hw-major for
mm2 (ftg), which kills the on-device PE transposes and frees a PSUM bank.

Scores land in two 2-bank PSUM tiles per group so the softplus pointwise
ops run on [128, 1024] (half the instruction count).  The weight pool
holds 12 tiles (3 groups of prefetch): deeper prefetch front-loads the
shared DMA engines and delays the output stores; shallower starves mm1.

DMA queues: weight/feature loads issue on SP (HWDGE) and output stores on
the otherwise-idle Pool engine (SWDGE), so a store waiting on its tile
never head-of-line blocks the weight prefetch stream.  PSUM->SBUF output
copies (the fp32->fp16 conversion) split over ACT and DVE (GPSIMD cannot
read PSUM).
"""

import numpy as np
from contextlib import ExitStack

import concourse.bass as bass
import concourse.bacc as bacc
import concourse.tile as tile
from concourse import mybir
from concourse.bass_utils import run_bass_kernel_spmd

F32 = mybir.dt.float32
F32R = mybir.dt.float32r
F16 = mybir.dt.float16
AF = mybir.ActivationFunctionType
ALU = mybir.AluOpType

N_CORES = 8
B_FULL, C, H, W, N = 256, 256, 4, 4, 2048
HW = H * W                  # 16
B = B_FULL // N_CORES       # 32 batches per core
KC = C // 128               # 2 contraction chunks of 128
GB = 4                      # batches per partition group (offsets 0/32/64/96)
GP = 32 * GB                # 128 partitions per group
NG = B // GB                # 8 groups per core
NCH = 4                     # n chunks per group chain
NW = N // NCH               # 512 (one PSUM bank)
NH = N // 2                 # 1024 (half-N weight tile)
CONST = 1e-4


def aux_inputs():
    # bd[p, j] = 1 iff row p is one of batch-slot j's real hw rows
    bd = np.zeros((GP, GB), np.float32)
    for p in range(GP):
        if p % 32 < HW:
            bd[p, p // 32] = 1.0
    # exp4[j, p] = 1 iff partition p belongs to batch-slot j's 32-block
    exp4 = np.zeros((GB, GP), np.float32)
    for p in range(GP):
        exp4[p // 32, p] = 1.0
    return {"bd": bd, "exp4": exp4}


def build_nc(debug=False):
    nc = bacc.Bacc(None, target_bir_lowering=False, debug=debug)
    feat = nc.dram_tensor("fpad", [128, KC, B, 32], F16, kind="ExternalInput")
    ftg_d = nc.dram_tensor("ftg", [NG, GP, KC, 128], F16, kind="ExternalInput")
    wts = nc.dram_tensor("weights", [B, C, N], F16, kind="ExternalInput")
    out = nc.dram_tensor("out", [B, C, N], F16, kind="ExternalOutput")
    bd_d = nc.dram_tensor("bd", [GP, GB], F32R, kind="ExternalInput")
    exp_d = nc.dram_tensor("exp4", [GB, GP], F32R, kind="ExternalInput")

    # [ci, b, kc, n] views of the DRAM tensors
    wts_r = wts.ap().rearrange("b (kc ci) n -> ci b kc n", kc=KC)
    out_r = out.ap().rearrange("b (kc ci) n -> ci b kc n", kc=KC)

    with tile.TileContext(nc) as tc, ExitStack() as ctx:
        singles = ctx.enter_context(tc.tile_pool(name="singles", bufs=1))
        wpool = ctx.enter_context(tc.tile_pool(name="w", bufs=12))
        opool = ctx.enter_context(tc.tile_pool(name="o", bufs=6))
        numpool = ctx.enter_context(tc.tile_pool(name="num", bufs=3))
        attpool = ctx.enter_context(tc.tile_pool(name="att", bufs=3))
        smallpool = ctx.enter_context(tc.tile_pool(name="small", bufs=4))
        ftpool = ctx.enter_context(tc.tile_pool(name="ft", bufs=6))
        ps_sc = ctx.enter_context(tc.tile_pool(name="ps_sc", bufs=2, space="PSUM"))
        ps_dr = ctx.enter_context(tc.tile_pool(name="ps_dr", bufs=2, space="PSUM"))
        ps_o = ctx.enter_context(tc.tile_pool(name="ps_o", bufs=2, space="PSUM"))

        bd_t = singles.tile([GP, GB], F32R)
        nc.sync.dma_start(out=bd_t, in_=bd_d.ap())
        exp_t = singles.tile([GB, GP], F32R)
        nc.sync.dma_start(out=exp_t, in_=exp_d.ap())

        # features, pre-transposed + hw-padded to 32 with zeros on the host
        f_t = singles.tile([128, KC, B, 32], F16)
        nc.sync.dma_start(out=f_t, in_=feat.ap())

        st = {}
        ev = 0

        def issue_loads(g):
            bs = list(range(g * GB, (g + 1) * GB))
            w_t = {}
            for b in bs:
                w_t[b] = wpool.tile([128, KC, N], F16, tag="w", name="w_t")
                nc.sync.dma_start(out=w_t[b], in_=wts_r[:, b])
            ft_t = ftpool.tile([GP, KC, 128], F16, name="ft_t")
            nc.sync.dma_start(out=ft_t, in_=ftg_d.ap()[g])
            st[g] = {"bs": bs, "ft": ft_t, "w": w_t}

        def issue_mm1(g):
            # scores land in two 2-bank PSUM tiles [128, 2, 512] so the
            # pointwise softplus ops run on [128, 1024] (half the op count)
            s = st[g]
            sc_l = [ps_sc.tile([GP, 2, NW], F32, name="sc_ps") for _ in range(2)]
            for nb in range(NCH):
                for j in range(GB):
                    for kc in range(KC):
                        nc.tensor.matmul(
                            sc_l[nb // 2][32 * j : 32 * j + 32, nb % 2, :],
                            f_t[:, kc, s["bs"][j], :],
                            s["w"][s["bs"][j]][:, kc, nb * NW : (nb + 1) * NW],
                            start=(kc == 0),
                            stop=(kc == KC - 1),
                            tile_position=(0, 32 * j),
                        )
            s["sc"] = sc_l

        def issue_softplus(g):
            # softplus(x) = max(x,0) + ln(1 + exp(-|x|)): exp arg <= 0 so no
            # overflow, Ln input stays in [1,2].  All Abs+Exp, then all Lns,
            # so the ACT table-set switches twice per group, not per chunk.
            s = st[g]
            te_l, num_l = [], []
            exp_insts = []
            for p in range(2):
                t_abs = numpool.tile([GP, 2, NW], F32, tag="tabs", bufs=2)
                nc.scalar.activation(t_abs, s["sc"][p], AF.Abs)
                t_exp = numpool.tile([GP, 2, NW], F32, tag="texp", bufs=2)
                exp_insts.append(
                    nc.scalar.activation(t_exp, t_abs, AF.Exp, scale=-1.0)
                )
                te_l.append(t_exp)
            for p in range(2):
                t_ln = numpool.tile([GP, 2, NW], F32, tag="tln", bufs=2)
                ln_i = nc.scalar.activation(t_ln, te_l[p], AF.Ln, bias=1.0)
                tile.add_dep_helper(
                    ln_i.ins, exp_insts[-1].ins, sync=False,
                    reason="cluster Lns after all Exps (one table switch)",
                )
                num_t = numpool.tile([GP, 2, NW], F32R, tag="num", bufs=2)
                nc.vector.scalar_tensor_tensor(
                    num_t, s["sc"][p], 0.0, t_ln, op0=ALU.max, op1=ALU.add
                )
                num_l.append(num_t)
            s["num"] = num_l

        def issue_att(g):
            s = st[g]
            att_t = attpool.tile([GP, NCH, NW], F16)
            for nb in range(NCH):
                num_nb = s["num"][nb // 2][:, nb % 2, :]
                d_ps = ps_dr.tile([GB, NW], F32, tag="dr", name="d_ps")
                nc.tensor.matmul(d_ps, bd_t, num_nb, start=True, stop=True)
                r_t = smallpool.tile([GB, NW], F32R)
                with nc.allow_low_precision(reason="tf32 matmul operand"):
                    nc.vector.tensor_scalar_add(r_t, d_ps, HW * CONST)
                    nc.vector.reciprocal(r_t, r_t)
                rb_ps = ps_dr.tile([GP, NW], F32, tag="dr", name="rb_ps")
                nc.tensor.matmul(rb_ps, exp_t, r_t, start=True, stop=True)
                # att = (num + CONST) * (1/denom), written fp16 for mm2
                with nc.allow_low_precision(reason="fp16 mm2 operand"):
                    nc.vector.scalar_tensor_tensor(
                        att_t[:, nb, :],
                        num_nb,
                        CONST,
                        rb_ps,
                        op0=ALU.add,
                        op1=ALU.mult,
                    )
            s["att"] = att_t

        def issue_mm2(g):
            # chunk-pair-major inner order: the first four matmuls of each
            # batch depend only on the first att pair
            s = st[g]
            att_t = s["att"]
            s["o_ps"] = []
            for j in range(GB):
                o_sb = opool.tile([128, KC, N], F16, tag="o", name="o_sb")
                for kc in range(KC):
                    for nb in range(NCH):
                        o_ps = ps_o.tile([128, NW], F32)
                        nc.tensor.matmul(
                            o_ps,
                            s["ft"][32 * j : 32 * j + HW, kc, :],
                            att_t[32 * j : 32 * j + HW, nb, :],
                            start=True,
                            stop=True,
                            tile_position=(32 * j, 0),
                        )
                        s["o_ps"].append((j, kc, nb, o_ps, o_sb))

        def issue_out(g):
            nonlocal ev
            s = st[g]
            for j, kc, nb, o_ps, o_sb in s["o_ps"]:
                dst = o_sb[:, kc, nb * NW : (nb + 1) * NW]
                with nc.allow_low_precision(reason="fp16 output"):
                    # GPSIMD can't read PSUM; ACT-heavy split since ACT
                    # copies are cheaper and DVE carries the denominator/
                    # attention elementwise chain.
                    if ev % 16 < 9:
                        nc.scalar.copy(dst, o_ps)
                    else:
                        nc.vector.tensor_copy(dst, o_ps)
                ev += 1
                if kc == KC - 1 and nb == NCH - 1:
                    nc.gpsimd.dma_start(out=out_r[:, s["bs"][j]], in_=o_sb)
            del st[g]

        # Pipeline: copies+stores of group g-1 issue AFTER softplus(g), so
        # the ACT queue never parks on an mm2-gated copy while softplus work
        # (which gates the whole denominator chain) is available.
        for g in range(NG):
            issue_loads(g)
            issue_mm1(g)
            if g >= 1:
                issue_att(g - 1)
                issue_mm2(g - 1)
            issue_softplus(g)
            if g >= 1:
                issue_out(g - 1)
        issue_att(NG - 1)
        issue_mm2(NG - 1)
        issue_out(NG - 1)

    nc.compile()
    return nc


_NC_CACHE = {}


def _get_nc():
    if "nc" not in _NC_CACHE:
        _NC_CACHE["nc"] = build_nc()
    return _NC_CACHE["nc"]


def prep_features(features):
    """[B_FULL, C, H, W] f32 -> (fpad [128, KC, B_FULL, 32] f16,
    ftg [B_FULL//GB, GP, KC, 128] f16)."""
    features = np.asarray(features, dtype=np.float32)
    f4 = features.reshape(B_FULL, KC, 128, HW)
    fpad = np.zeros((B_FULL, KC, 128, 32), np.float16)
    fpad[..., :HW] = f4
    fpad = np.ascontiguousarray(fpad.transpose(2, 1, 0, 3))  # [128, KC, b, 32]
    # hw-major for mm2: ftg[g, 32j+r, kc, ci] = f[4g+j, kc*128+ci, r]
    ftp = np.zeros((B_FULL, 32, KC, 128), np.float16)
    ftp[:, :HW] = f4.transpose(0, 3, 1, 2)
    ftg = np.ascontiguousarray(ftp.reshape(B_FULL // GB, GP, KC, 128))
    return fpad, ftg


def run(features, weights, trace=False, **kwargs):
    """Shard over 8 cores, run, gather. Returns (out, BassKernelResults)."""
    fpad, ftg = prep_features(features)
    weights = np.asarray(weights).astype(np.float16)
    aux = aux_inputs()
    nc = _get_nc()
    in_maps = []
    for i in range(N_CORES):
        sl = slice(i * B, (i + 1) * B)
        gl = slice(i * NG, (i + 1) * NG)
        in_maps.append(
            {
                "fpad": np.ascontiguousarray(fpad[:, :, sl]),
                "ftg": ftg[gl],
                "weights": weights[sl],
                **aux,
            }
        )
    res = run_bass_kernel_spmd(
        nc, in_maps, core_ids=list(range(N_CORES)), trace=trace, **kwargs
    )
    out = np.concatenate([r["out"] for r in res.results], axis=0).astype(np.float32)
    return out, res


def kernel(features, weights):
    out, _ = run(features, weights)
    return out


# revision 29
# speedup vs baseline: 1.0335x; 1.0329x over previous
"""Attentional pooling layer on Trainium2 (Bass/Tile), 8-core batch-parallel.

Reference computation per batch b:
    scores[hw, n] = sum_c f[c, hw] * w[c, n]          (mm1, fp16 in, f32 acc)
    num           = softplus(scores)                  (ACT: Abs/Exp/Ln, f32)
    denom[n]      = sum_hw num[hw, n] + 16*CONST      (PE reduce + DVE)
    att[hw, n]    = (num + CONST) / denom[n]          (PE bcast + DVE, fp16)
    out[c, n]     = sum_hw f[c, hw] * att[hw, n]      (mm2, fp16 in, f32 acc)

The problem is HBM-bandwidth-bound: per core 32 batches x 2 MiB of weights
in and 2 MiB of outputs out.  Both streams run in fp16 (tolerance is 2e-2;
fp16 end-to-end lands ~5e-4), halving DMA traffic vs fp32.  Weights are
converted to fp16 on the host; outputs are written fp16 by the PSUM->SBUF
copy and upconverted on the host.

Partition layout: 4 batches are packed into one full 128-partition group at
32-partition offsets (fp16 matmuls accept explicit tile_position col bases
0/32/64/96, unlike f32r which requires base 0).  32 batches per core = 8
exact groups of 4.  Partition-dim reductions (sum over hw) and broadcasts
(denom over hw) are tiny constant 0/1 f32r matmuls (bd / exp4) fed from
host numpy.  Features arrive twice from the host, both fp16 and tiny: once
c-major zero-padded for mm1 (fpad) and once pre-transposed hw-major for
mm2 (ftg), which kills the on-device PE transposes and frees a PSUM bank.

Scores land in two 2-bank PSUM tiles per group so the softplus pointwise
ops run on [128, 1024] (half the instruction count).  The weight pool
holds 12 tiles (3 groups of prefetch): deeper prefetch front-loads the
shared DMA engines and delays the output stores; shallower starves mm1.

DMA queues: weight/feature loads issue on SP (HWDGE) and output stores on
the otherwise-idle Pool engine (SWDGE), so a store waiting on its tile
never head-of-line blocks the weight prefetch stream.  PSUM->SBUF output
copies (the fp32->fp16 conversion) split over ACT and DVE (GPSIMD cannot
read PSUM).
"""

import numpy as np
from contextlib import ExitStack

import concourse.bass as bass
import concourse.bacc as bacc
import concourse.tile as tile
from concourse import mybir
from concourse.bass_utils import run_bass_kernel_spmd

F32 = mybir.dt.float32
F32R = mybir.dt.float32r
F16 = mybir.dt.float16
AF = mybir.ActivationFunctionType
ALU = mybir.AluOpType

N_CORES = 8
B_FULL, C, H, W, N = 256, 256, 4, 4, 2048
HW = H * W                  # 16
B = B_FULL // N_CORES       # 32 batches per core
KC = C // 128               # 2 contraction chunks of 128
GB = 4                      # batches per partition group (offsets 0/32/64/96)
GP = 32 * GB                # 128 partitions per group
NG = B // GB                # 8 groups per core
NCH = 4                     # n chunks per group chain
NW = N // NCH               # 512 (one PSUM bank)
NH = N // 2                 # 1024 (half-N weight tile)
CONST = 1e-4


def aux_inputs():
    # bd[p, j] = 1 iff row p is one of batch-slot j's real hw rows
    bd = np.zeros((GP, GB), np.float32)
    for p in range(GP):
        if p % 32 < HW:
            bd[p, p // 32] = 1.0
    # exp4[j, p] = 1 iff partition p belongs to batch-slot j's 32-block
    exp4 = np.zeros((GB, GP), np.float32)
    for p in range(GP):
        exp4[p // 32, p] = 1.0
    return {"bd": bd, "exp4": exp4}


def build_nc(debug=False):
    nc = bacc.Bacc(None, target_bir_lowering=False, debug=debug)
    feat = nc.dram_tensor("fpad", [128, KC, B, 32], F16, kind="ExternalInput")
    ftg_d = nc.dram_tensor("ftg", [NG, GP, KC, 128], F16, kind="ExternalInput")
    wts = nc.dram_tensor("weights", [B, C, N], F16, kind="ExternalInput")
    out = nc.dram_tensor("out", [B, C, N], F16, kind="ExternalOutput")
    bd_d = nc.dram_tensor("bd", [GP, GB], F32R, kind="ExternalInput")
    exp_d = nc.dram_tensor("exp4", [GB, GP], F32R, kind="ExternalInput")

    # [ci, b, kc, (h n)] views of the DRAM tensors (h = half of N)
    wts_r = wts.ap().rearrange("b (kc ci) (h n) -> ci b kc h n", kc=KC, h=2)
    out_r = out.ap().rearrange("b (kc ci) (h n) -> ci b kc h n", kc=KC, h=2)

    with tile.TileContext(nc) as tc, ExitStack() as ctx:
        singles = ctx.enter_context(tc.tile_pool(name="singles", bufs=1))
        wpool = ctx.enter_context(tc.tile_pool(name="w", bufs=16))
        opool = ctx.enter_context(tc.tile_pool(name="o", bufs=8))
        numpool = ctx.enter_context(tc.tile_pool(name="num", bufs=3))
        attpool = ctx.enter_context(tc.tile_pool(name="att", bufs=3))
        smallpool = ctx.enter_context(tc.tile_pool(name="small", bufs=4))
        ftpool = ctx.enter_context(tc.tile_pool(name="ft", bufs=6))
        ps_sc = ctx.enter_context(tc.tile_pool(name="ps_sc", bufs=2, space="PSUM"))
        ps_dr = ctx.enter_context(tc.tile_pool(name="ps_dr", bufs=1, space="PSUM"))
        ps_o = ctx.enter_context(tc.tile_pool(name="ps_o", bufs=2, space="PSUM"))

        bd_t = singles.tile([GP, GB], F32R)
        nc.sync.dma_start(out=bd_t, in_=bd_d.ap())
        exp_t = singles.tile([GB, GP], F32R)
        nc.sync.dma_start(out=exp_t, in_=exp_d.ap())

        # features, pre-transposed + hw-padded to 32 with zeros on the host
        f_t = singles.tile([128, KC, B, 32], F16)
        nc.sync.dma_start(out=f_t, in_=feat.ap())

        st = {}
        ev = 0
        NU = 2 * NG  # pipeline unit = (group, half of N); 16 units

        def issue_loads(u):
            g, h = u // 2, u % 2
            bs = list(range(g * GB, (g + 1) * GB))
            w_t = {}
            for b in bs:
                w_t[b] = wpool.tile([128, KC, NH], F16, tag="w", name="w_t")
                nc.sync.dma_start(out=w_t[b], in_=wts_r[:, b, :, h])
            s = {"bs": bs, "w": w_t, "g": g, "h": h}
            if h == 0:
                ft_t = ftpool.tile([GP, KC, 128], F16, name="ft_t")
                nc.sync.dma_start(out=ft_t, in_=ftg_d.ap()[g])
                s["ft"] = ft_t
            else:
                s["ft"] = st[u - 1]["ft"]
            st[u] = s

        def issue_mm1(u):
            # both 512-chunks of this half land in one 2-bank PSUM tile so
            # the pointwise softplus ops run on [128, 1024]
            s = st[u]
            sc = ps_sc.tile([GP, 2, NW], F32, name="sc_ps")
            for q in range(2):
                for j in range(GB):
                    for kc in range(KC):
                        nc.tensor.matmul(
                            sc[32 * j : 32 * j + 32, q, :],
                            f_t[:, kc, s["bs"][j], :],
                            s["w"][s["bs"][j]][:, kc, q * NW : (q + 1) * NW],
                            start=(kc == 0),
                            stop=(kc == KC - 1),
                            tile_position=(0, 32 * j),
                        )
            s["sc"] = sc

        def issue_softplus(u):
            # softplus(x) = max(x,0) + ln(1 + exp(-|x|)): exp arg <= 0 so no
            # overflow, Ln input stays in [1,2]
            s = st[u]
            t_abs = numpool.tile([GP, 2, NW], F32, tag="tabs", bufs=2)
            nc.scalar.activation(t_abs, s["sc"], AF.Abs)
            t_exp = numpool.tile([GP, 2, NW], F32, tag="texp", bufs=2)
            nc.scalar.activation(t_exp, t_abs, AF.Exp, scale=-1.0)
            t_ln = numpool.tile([GP, 2, NW], F32, tag="tln", bufs=2)
            nc.scalar.activation(t_ln, t_exp, AF.Ln, bias=1.0)
            num_t = numpool.tile([GP, 2, NW], F32R, tag="num", bufs=2)
            nc.vector.scalar_tensor_tensor(
                num_t, s["sc"], 0.0, t_ln, op0=ALU.max, op1=ALU.add
            )
            s["num"] = num_t

        def issue_att(u):
            # denominator chain on [*, 1024]: one 2-bank dr slot alternates
            # between d2 (reduce out) and rb2 (bcast out)
            s = st[u]
            att_t = attpool.tile([GP, 2, NW], F16)
            d2 = ps_dr.tile([GB, 2, NW], F32, tag="dr", name="d2")
            for q in range(2):
                nc.tensor.matmul(
                    d2[:, q, :], bd_t, s["num"][:, q, :], start=True, stop=True
                )
            r_t = smallpool.tile([GB, 2, NW], F32R)
            with nc.allow_low_precision(reason="tf32 matmul operand"):
                nc.vector.tensor_scalar_add(r_t, d2, HW * CONST)
                nc.vector.reciprocal(r_t, r_t)
            rb2 = ps_dr.tile([GP, 2, NW], F32, tag="dr", name="rb2")
            for q in range(2):
                nc.tensor.matmul(
                    rb2[:, q, :], exp_t, r_t[:, q, :], start=True, stop=True
                )
            # att = (num + CONST) * (1/denom), written fp16 for mm2
            with nc.allow_low_precision(reason="fp16 mm2 operand"):
                nc.vector.scalar_tensor_tensor(
                    att_t, s["num"], CONST, rb2, op0=ALU.add, op1=ALU.mult
                )
            s["att"] = att_t

        def issue_mm2(u):
            s = st[u]
            att_t = s["att"]
            s["o_ps"] = []
            for j in range(GB):
                o_sb = opool.tile([128, KC, NH], F16, tag="o", name="o_sb")
                for kc in range(KC):
                    for q in range(2):
                        o_ps = ps_o.tile([128, NW], F32)
                        nc.tensor.matmul(
                            o_ps,
                            s["ft"][32 * j : 32 * j + HW, kc, :],
                            att_t[32 * j : 32 * j + HW, q, :],
                            start=True,
                            stop=True,
                            tile_position=(32 * j, 0),
                        )
                        s["o_ps"].append((j, kc, q, o_ps, o_sb))

        def issue_out(u):
            nonlocal ev
            s = st[u]
            for j, kc, q, o_ps, o_sb in s["o_ps"]:
                dst = o_sb[:, kc, q * NW : (q + 1) * NW]
                with nc.allow_low_precision(reason="fp16 output"):
                    # GPSIMD can't read PSUM; ACT-heavy split since ACT
                    # copies are cheaper and DVE carries the denominator/
                    # attention elementwise chain.
                    if ev % 16 < 8:
                        nc.scalar.copy(dst, o_ps)
                    else:
                        nc.vector.tensor_copy(dst, o_ps)
                ev += 1
                if kc == KC - 1 and q == 1:
                    nc.gpsimd.dma_start(
                        out=out_r[:, s["bs"][j], :, s["h"]], in_=o_sb
                    )
            del st[u]

        # Pipeline over 16 half-group units: copies+stores of unit u-1 issue
        # AFTER softplus(u), so the ACT queue never parks on an mm2-gated
        # copy while softplus work (which gates the denom chain) is
        # available.
        for u in range(NU):
            issue_loads(u)
            issue_mm1(u)
            if u >= 1:
                issue_att(u - 1)
                issue_mm2(u - 1)
            issue_softplus(u)
            if u >= 1:
                issue_out(u - 1)
        issue_att(NU - 1)
        issue_mm2(NU - 1)
        issue_out(NU - 1)

    nc.compile()
    return nc


_NC_CACHE = {}


def _get_nc():
    if "nc" not in _NC_CACHE:
        _NC_CACHE["nc"] = build_nc()
    return _NC_CACHE["nc"]


def prep_features(features):
    """[B_FULL, C, H, W] f32 -> (fpad [128, KC, B_FULL, 32] f16,
    ftg [B_FULL//GB, GP, KC, 128] f16)."""
    features = np.asarray(features, dtype=np.float32)
    f4 = features.reshape(B_FULL, KC, 128, HW)
    fpad = np.zeros((B_FULL, KC, 128, 32), np.float16)
    fpad[..., :HW] = f4
    fpad = np.ascontiguousarray(fpad.transpose(2, 1, 0, 3))  # [128, KC, b, 32]
    # hw-major for mm2: ftg[g, 32j+r, kc, ci] = f[4g+j, kc*128+ci, r]
    ftp = np.zeros((B_FULL, 32, KC, 128), np.float16)
    ftp[:, :HW] = f4.transpose(0, 3, 1, 2)
    ftg = np.ascontiguousarray(ftp.reshape(B_FULL // GB, GP, KC, 128))
    return fpad, ftg


def run(features, weights, trace=False, **kwargs):
    """Shard over 8 cores, run, gather. Returns (out, BassKernelResults)."""
    fpad, ftg = prep_features(features)
    weights = np.asarray(weights).astype(np.float16)
    aux = aux_inputs()
    nc = _get_nc()
    in_maps = []
    for i in range(N_CORES):
        sl = slice(i * B, (i + 1) * B)
        gl = slice(i * NG, (i + 1) * NG)
        in_maps.append(
            {
                "fpad": np.ascontiguousarray(fpad[:, :, sl]),
                "ftg": ftg[gl],
                "weights": weights[sl],
                **aux,
            }
        )
    res = run_bass_kernel_spmd(
        nc, in_maps, core_ids=list(range(N_CORES)), trace=trace, **kwargs
    )
    out = np.concatenate([r["out"] for r in res.results], axis=0).astype(np.float32)
    return out, res


def kernel(features, weights):
    out, _ = run(features, weights)
    return out
